# revision 3
# baseline (speedup 1.0000x reference)
"""GQA kernel for Trainium2, 8 NeuronCores — v2.

Sharding: core c -> batch b = c//4, kv-head-group g = c%4.
Each core: 1 batch, 2 KV heads (2g, 2g+1), 8 Q heads (8g..8g+7),
row-shard of W_o (rows 512g..512g+512). Host sums 4 partials + bo.

v2 vs v1 (decisions driven by the TimelineSim cost model):
- x and all weights shipped bf16, DMA'd straight into matmul operands
  (no f32->f32r SBUF retag copies; halves input DMA).
- V projected directly into [t, d] layout (x chunk as stationary) —
  no PE transposes / ACT copies on the V path.
- Attention per 256-query strip: scores (kt f32r stationary, qt bf16
  moving), key-blocks batched 4-per-PSUM-tile so exp runs as one wide
  ACT instruction; A@V in swapped orientation (out [q, d+1], moving V
  bf16, N=65) so the softmax denominator lands in a per-partition
  column; normalize is one DVE divide; attn transposed back for the
  O-projection with a bf16-identity PE transpose.
- O-projection and later-stage projections emitted as PE filler units
  inside the ACT-bound attention stream (engine-frontier based pump).
"""

import numpy as np

E = 2048
S = 2048
B = 2
D = 64
NCORE = 8
EC = E // 128      # 16 contraction chunks
NKB = S // 128     # 16 key blocks
NST = 4            # x super-strips (512 t each)
NQS = 8            # query strips of 256

_CACHE = {}
# tile jb holds q-heads (jb, jb+4): kv0 head at partitions 0:64,
# kv1 head at partitions 64:128, matching the K/V partition layout
HEAD_PERM = [0, 4, 1, 5, 2, 6, 3, 7]

PEC = 1e9 / 2.4e9  # ns per PE cycle (schedule heuristic only)


def _build():
    import concourse.tile as tile
    from concourse import mybir, bacc
    from concourse.masks import make_identity

    F32 = mybir.dt.float32
    F32R = mybir.dt.float32r
    BF16 = mybir.dt.bfloat16
    Exp = mybir.ActivationFunctionType.Exp
    DIV = mybir.AluOpType.divide
    LE = mybir.AluOpType.is_le
    GE = mybir.AluOpType.is_ge
    POW = mybir.AluOpType.pow
    MUL = mybir.AluOpType.mult

    nc = bacc.Bacc("TRN2", target_bir_lowering=False, debug=False,
                   num_devices=NCORE)

    XT = nc.declare_dram_parameter("xt", [E, S], BF16, isOutput=False)
    WQ = nc.declare_dram_parameter("wq", [128, EC, 512], BF16, isOutput=False)
    WK = nc.declare_dram_parameter("wk", [128, EC, 128], BF16, isOutput=False)
    WV = nc.declare_dram_parameter("wv", [128, EC, 128], BF16, isOutput=False)
    WO = nc.declare_dram_parameter("wo", [128, 4, E], BF16, isOutput=False)
    BIAS = nc.declare_dram_parameter("bias", [128, 5], F32, isOutput=False)
    BVT = nc.declare_dram_parameter("bvt", [128, 128], F32, isOutput=False)
    OUT = nc.declare_dram_parameter("out", [S, E], BF16, isOutput=True)

    with tile.TileContext(nc) as tc:
        with tc.tile_pool(name="persist", bufs=1) as persist, \
             tc.tile_pool(name="xr", bufs=3) as xrp, \
             tc.tile_pool(name="at", bufs=34) as atp, \
             tc.tile_pool(name="aqd", bufs=6) as aqdp, \
             tc.tile_pool(name="attn", bufs=32) as attnp, \
             tc.tile_pool(name="ost", bufs=2) as ostp, \
             tc.tile_pool(name="st", bufs=2, space="PSUM") as stp, \
             tc.tile_pool(name="av", bufs=2, space="PSUM") as avp, \
             tc.tile_pool(name="op", bufs=2, space="PSUM") as opp:

            wq_t = persist.tile([128, EC, 512], BF16, tag="wq")
            wk_t = persist.tile([128, EC, 128], BF16, tag="wk")
            wv_t = persist.tile([128, EC, 128], BF16, tag="wv")
            wo_t = persist.tile([128, 4, E], BF16, tag="wo")
            qt = [persist.tile([128, S], BF16, tag=f"qt{j}", name=f"qt{j}")
                  for j in range(4)]
            kt = persist.tile([128, S], BF16, tag="kt")
            v_t = persist.tile([128, NKB, 2, D + 1], BF16, tag="vt")
            bvt = persist.tile([128, 128], F32, tag="bvt")
            bias_t = persist.tile([128, 5], F32, tag="bias")
            ident = persist.tile([128, 128], BF16, tag="ident")

            make_identity(nc, ident)
            nc.vector.memset(v_t[:, :, :, D:D + 1], 1.0)

            xt_view = XT[:, :].rearrange("(ec p) t -> p ec t", p=128)
            xr = [xrp.tile([128, EC, 512], BF16, tag="xr", name=f"x{sg}")
                  for sg in range(NST)]
            # DMA order matters: the DMA engines drain roughly in emission
            # order, and the first K/V/Q projections need bias+x0+wk+wv+wq.
            # wo/bvt/cmt are not needed until well into the run.
            nc.sync.dma_start(out=bias_t, in_=BIAS[:, :])
            nc.sync.dma_start(out=wk_t, in_=WK[:, :, :])
            nc.sync.dma_start(out=xr[0][:, 0:EC // 2, :],
                              in_=xt_view[:, 0:EC // 2, 0:512])
            nc.sync.dma_start(out=xr[0][:, EC // 2:, :],
                              in_=xt_view[:, EC // 2:, 0:512])
            nc.sync.dma_start(out=wv_t, in_=WV[:, :, :])
            for jb in range(4):
                nc.sync.dma_start(out=wq_t[:, :, jb * 128:(jb + 1) * 128],
                                  in_=WQ[:, :, jb * 128:(jb + 1) * 128])
            nc.sync.dma_start(out=bvt, in_=BVT[:, :])
            nc.sync.dma_start(out=xr[1],
                              in_=xt_view[:, :, 512:1024])
            for sg in range(2, NST):
                nc.sync.dma_start(out=xr[sg],
                                  in_=xt_view[:, :, sg * 512:(sg + 1) * 512])
            nc.sync.dma_start(out=wo_t, in_=WO[:, :, :])

            # ---- schedule bookkeeping: emitted-work frontiers ----
            sched = {"pe": 0.0, "act": 0.0, "dve": 0.0, "pool": 0.0}
            fill_o = []    # (tb, fn)
            fill_kvq = []  # (sg, fn)

            def pump_one():
                # prefer projection units (needed earliest); O-projection
                # units are saved as late filler for the ACT-bound tail
                if fill_kvq:
                    _, fn = fill_kvq.pop(0)
                elif fill_o:
                    _, fn = fill_o.pop(0)
                else:
                    return False
                sched["pe"] += fn()
                return True

            import os
            dbg = os.environ.get("SCHED_DEBUG")
            starve = {"ns": 0.0, "n": 0}

            def pump_until(t_ns):
                while sched["pe"] < t_ns:
                    if not pump_one():
                        starve["ns"] += t_ns - sched["pe"]
                        starve["n"] += 1
                        break

            def pump_force(sg_max, tb_max):
                while fill_kvq and fill_kvq[0][0] <= sg_max:
                    _, fn = fill_kvq.pop(0)
                    sched["pe"] += fn()
                while fill_o and fill_o[0][0] <= tb_max:
                    _, fn = fill_o.pop(0)
                    sched["pe"] += fn()

            # ---- projection units ----
            def k_unit(sg):
                def fn():
                    ps = opp.tile([128, 512], F32, tag="op", name="kps")
                    for ec in range(EC):
                        nc.tensor.matmul(ps, wk_t[:, ec, :], xr[sg][:, ec, :],
                                         start=(ec == 0), stop=(ec == EC - 1),
                                         skip_group_check=True)
                    nc.vector.tensor_scalar_add(
                        kt[:, sg * 512:(sg + 1) * 512], ps, bias_t[:, 4:5])
                    return 16 * 512 * PEC
                return fn

            def v_unit(sg, i):
                tb = 4 * sg + i

                def fn():
                    ps = opp.tile([128, 512], F32, tag="op", name="vps")
                    for ec in range(EC):
                        nc.tensor.matmul(ps[:, 0:128],
                                         xr[sg][:, ec, i * 128:(i + 1) * 128],
                                         wv_t[:, ec, :],
                                         start=(ec == 0), stop=(ec == EC - 1),
                                         skip_group_check=True)
                    for kv in range(2):
                        nc.vector.tensor_add(
                            v_t[:, tb, kv, 0:D],
                            ps[:, kv * 64:kv * 64 + 64],
                            bvt[:, kv * 64:kv * 64 + 64])
                    return 16 * 128 * PEC
                return fn

            def q_unit(sg, jb):
                def fn():
                    ps = opp.tile([128, 512], F32, tag="op", name="qps")
                    for ec in range(EC):
                        nc.tensor.matmul(ps,
                                         wq_t[:, ec, jb * 128:(jb + 1) * 128],
                                         xr[sg][:, ec, :],
                                         start=(ec == 0), stop=(ec == EC - 1),
                                         skip_group_check=True)
                    nc.vector.tensor_scalar_add(
                        qt[jb][:, sg * 512:(sg + 1) * 512], ps,
                        bias_t[:, jb:jb + 1])
                    return 16 * 512 * PEC
                return fn

            def kvq_units(sg):
                out = [(sg, k_unit(sg))]
                out += [(sg, v_unit(sg, i)) for i in range(4)]
                out += [(sg, q_unit(sg, jb)) for jb in range(4)]
                return out

            # ---- O-projection units ----
            ostage = {}

            def o_unit(tb, ng, tiles):
                def fn():
                    if ng == 0:
                        ostage[tb] = ostp.tile([128, E], BF16, tag="ost",
                                               name=f"ost{tb}")
                    ps = opp.tile([128, 512], F32, tag="op", name="ops")
                    for jc in range(4):
                        nc.tensor.matmul(ps, tiles[jc],
                                         wo_t[:, jc, ng * 512:(ng + 1) * 512],
                                         start=(jc == 0), stop=(jc == 3),
                                         skip_group_check=True)
                    nc.vector.tensor_copy(
                        ostage[tb][:, ng * 512:(ng + 1) * 512], ps)
                    sched["dve"] += 658
                    if ng == 3:
                        nc.sync.dma_start(
                            out=OUT[tb * 128:(tb + 1) * 128, :],
                            in_=ostage[tb])
                    return 4 * 512 * PEC
                return fn

            # ---- prologue: stage-0 projections inline ----
            for _, fn in kvq_units(0):
                sched["pe"] += fn()
            fill_kvq.extend(kvq_units(1))

            # ---- attention ----
            # Cross-unit software pipeline: the A@V + normalize work of
            # unit u is deferred into a pending queue and emitted while
            # unit u+1's scores/exp stream runs, so the ACT (exp) lane is
            # never delayed by PE work that could wait.
            pending = []  # (pe_cost_ns, fn, tile_seq) in dependency order
            tile_seq = {"n": 0}
            drain_thr = [28]

            def drain_old(cur_uid):
                # emit one deferred attention task at least 3 exp-tiles old,
                # so its exp has had time to complete on its engine
                if pending and pending[0][2] <= cur_uid - drain_thr[0]:
                    c, fn, _ = pending.pop(0)
                    fn()
                    sched["pe"] += c
                    return True
                return False

            def drain_until(t_ns, cur_uid):
                while sched["pe"] < t_ns:
                    if drain_old(cur_uid):
                        continue
                    if not pump_one():
                        starve["ns"] += t_ns - sched["pe"]
                        starve["n"] += 1
                        break

            def drain_pending():
                while pending:
                    c, fn, _ = pending.pop(0)
                    fn()
                    sched["pe"] += c

            attn_tiles = {}

            for qsg in range(NQS):
                q0 = qsg * 256
                if qsg >= 2 and qsg % 2 == 0:
                    sg = qsg // 2
                    pump_force(sg, 2 * qsg - 11)
                    if sg + 1 < NST:
                        fill_kvq.extend(kvq_units(sg + 1))

                if dbg:
                    print(f"qsg{qsg}: pe={sched['pe']/1000:.1f}us "
                          f"act={sched['act']/1000:.1f}us "
                          f"q_kvq={len(fill_kvq)} q_o={len(fill_o)} "
                          f"pend={len(pending)} "
                          f"starved={starve['ns']/1000:.1f}us/{starve['n']}")
                drain_thr[0] = 2 if qsg < 2 else 32
                n_kb = 2 * qsg + 2
                kb_diag = 2 * qsg
                kb_narrow = 2 * qsg + 1
                G = (n_kb + 3) // 4

                def av_task(at, kbs, av, kv, kbd, kbn):
                    # start_tensor_calc pending-zeroes the WHOLE 2KB psum
                    # bank, so the four accumulation regions sharing this av
                    # bank must use exactly ONE start (chronologically first
                    # matmul, kv0/kb0/qb0) and ONE stop (last, kv1/kbn/qb1).
                    def fn():
                        for kb in kbs:
                            s = kb % 4
                            if kb != kbn:
                                nc.tensor.matmul(
                                    av[:, 0, kv * 65:kv * 65 + 65],
                                    at[:, 256 * s:256 * s + 128],
                                    v_t[:, kb, kv, :],
                                    start=(kv == 0 and kb == 0), stop=False,
                                    skip_group_check=True)
                                nc.tensor.matmul(
                                    av[:, 1, kv * 65:kv * 65 + 65],
                                    at[:, 256 * s + 128:256 * s + 256],
                                    v_t[:, kb, kv, :],
                                    start=False, stop=False,
                                    skip_group_check=True)
                            else:
                                nc.tensor.matmul(
                                    av[:, 1, kv * 65:kv * 65 + 65],
                                    at[:, 256 * s:256 * s + 128],
                                    v_t[:, kb, kv, :],
                                    start=False,
                                    stop=(kv == 1 and kb == kbn),
                                    skip_group_check=True)
                    cost = sum(65 * (1 if kb == kbn else 2) for kb in kbs)
                    return (cost * PEC, fn)

                def norm_task(hh, av, qsg_, last_hh):
                    def fn():
                        for qb in range(2):
                            aqd = aqdp.tile([128, 128], BF16, tag="aqd",
                                            name="aqd")
                            linv = aqdp.tile([128, 2], F32, tag="linv",
                                             name="linv")
                            nc.vector.reciprocal(
                                linv, av[:, qb, D::D + 1])
                            for kv in range(2):
                                base = kv * (D + 1)
                                nc.vector.tensor_scalar_mul(
                                    aqd[:, kv * 64:(kv + 1) * 64],
                                    av[:, qb, base:base + D],
                                    linv[:, kv:kv + 1])
                            tpt = opp.tile([128, 512], F32, tag="op",
                                           name="tp")
                            tpv = tpt.bitcast(BF16)[:, 0:128]
                            nc.tensor.transpose(tpv, aqd, ident)
                            asb = attnp.tile([128, 128], BF16, tag="attn",
                                             name=f"a{hh}")
                            nc.vector.tensor_copy(asb, tpv)
                            attn_tiles[(2 * qsg_ + qb, hh)] = asb
                        sched["dve"] += 4 * 190 + 2 * 190
                        if last_hh:
                            for qb in range(2):
                                tb = 2 * qsg_ + qb
                                tiles = [attn_tiles[(tb, jc)]
                                         for jc in range(4)]
                                for ng in range(4):
                                    fill_o.append((tb, o_unit(tb, ng, tiles)))
                    return (2 * 128 * PEC, fn)

                for hh in range(4):
                    av = avp.tile([128, 2, 2 * (D + 1)], F32, tag="av",
                                  name=f"av{hh}")
                    for kv in range(2):
                        uid = qsg * 8 + hh * 2 + kv
                        qoff = kv * 64
                        for g in range(G):
                            kbs = list(range(4 * g, min(4 * g + 4, n_kb)))
                            st = stp.tile([128, 1024], F32, tag="st",
                                          name="st")
                            w = 0
                            for kb in kbs:
                                s = kb % 4
                                if kb == kb_narrow:
                                    cols = qt[hh][qoff:qoff + 64,
                                                  q0 + 128:q0 + 256]
                                    dst = st[:, 256 * s:256 * s + 128]
                                    w = 256 * s + 128
                                    sched["pe"] += 128 * PEC
                                else:
                                    cols = qt[hh][qoff:qoff + 64,
                                                  q0:q0 + 256]
                                    dst = st[:, 256 * s:256 * s + 256]
                                    w = 256 * s + 256
                                    sched["pe"] += 256 * PEC
                                nc.tensor.matmul(
                                    dst,
                                    kt[qoff:qoff + 64,
                                       kb * 128:(kb + 1) * 128],
                                    cols, start=True, stop=True,
                                    skip_group_check=True)
                            at = atp.tile([128, 1024], BF16, tag="at",
                                          name="at")
                            eng = "act"
                            exp_ns = w * 0.8333 + 370
                            nc.scalar.activation(at[:, 0:w], st[:, 0:w], Exp)
                            sched[eng] = max(sched[eng],
                                             sched["pe"] + 150) + exp_ns
                            # causal zero-mask on diag blocks, post-exp, on
                            # the exp engine itself (DVE for ACT-exp'd tiles)
                            for kb in kbs:
                                if kb in (kb_diag, kb_narrow):
                                    s = kb % 4
                                    sl = at[:, 256 * s:256 * s + 128]
                                    nc.gpsimd.affine_select(
                                        out=sl, in_=sl,
                                        compare_op=GE, fill=0.0, base=0,
                                        pattern=[[1, 128]],
                                        channel_multiplier=-1)
                                    sched["pool"] = max(
                                        sched["pool"], sched[eng]) + 273
                            c, fn = av_task(at, kbs, av, kv,
                                            kb_diag, kb_narrow)
                            tile_seq["n"] += 1
                            pending.append((c, fn, tile_seq["n"]))
                            drain_old(tile_seq["n"])
                            drain_until(sched[eng] + 800, tile_seq["n"])
                    c, fn = norm_task(hh, av, qsg, hh == 3)
                    pending.append((c, fn, tile_seq["n"]))

            drain_pending()
            if dbg:
                print(f"end: pe={sched['pe']/1000:.1f}us act={sched['act']/1000:.1f}us "
                      f"starved={starve['ns']/1000:.1f}us/{starve['n']}")
            pump_force(NST, NKB)

    nc.compile()
    return nc


def _prep_core_inputs(c, x, Wq, bq, Wk, bk, Wv, bv, Wo, np_bf16, xt_cache):
    g = c % 4
    b = c // 4
    if b not in xt_cache:
        xt_cache[b] = np.ascontiguousarray(x[b].T).astype(np_bf16)
    wq_s = Wq[:, 512 * g:512 * (g + 1)].reshape(E, 8, 64)
    wq_s = wq_s[:, HEAD_PERM, :].reshape(E, 512) * np.float32(0.125)
    wq = np.ascontiguousarray(
        wq_s.reshape(EC, 128, 512).transpose(1, 0, 2)).astype(np_bf16)
    wk = np.ascontiguousarray(
        Wk[:, 128 * g:128 * (g + 1)].reshape(EC, 128, 128)
        .transpose(1, 0, 2)).astype(np_bf16)
    wv = np.ascontiguousarray(
        Wv[:, 128 * g:128 * (g + 1)].reshape(EC, 128, 128)
        .transpose(1, 0, 2)).astype(np_bf16)
    wo_s = Wo[512 * g:512 * (g + 1), :].reshape(8, 64, E)
    wo_s = wo_s[HEAD_PERM, :, :].reshape(512, E)
    wo = np.ascontiguousarray(
        wo_s.reshape(4, 128, E).transpose(1, 0, 2)).astype(np_bf16)
    bias = np.zeros((128, 5), np.float32)
    bq_s = bq[512 * g:512 * (g + 1)].reshape(8, 64)[HEAD_PERM, :].reshape(512)
    bias[:, 0:4] = bq_s.reshape(4, 128).T * 0.125
    bias[:, 4] = bk[128 * g:128 * (g + 1)]
    bvt = np.tile(bv[128 * g:128 * (g + 1)][None, :], (128, 1)).astype(
        np.float32)
    return {"xt": xt_cache[b], "wq": wq, "wk": wk, "wv": wv, "wo": wo,
            "bias": bias, "bvt": bvt}


def kernel(**inputs):
    from concourse.bass_utils import run_bass_kernel_spmd
    from concourse import mybir

    np_bf16 = mybir.dt.np(mybir.dt.bfloat16)

    x = np.asarray(inputs["x"], np.float32)
    Wq = np.asarray(inputs["Wq"], np.float32)
    bq = np.asarray(inputs["bq"], np.float32)
    Wk = np.asarray(inputs["Wk"], np.float32)
    bk = np.asarray(inputs["bk"], np.float32)
    Wv = np.asarray(inputs["Wv"], np.float32)
    bv = np.asarray(inputs["bv"], np.float32)
    Wo = np.asarray(inputs["Wo"], np.float32)
    bo = np.asarray(inputs["bo"], np.float32)

    if "nc" not in _CACHE:
        _CACHE["nc"] = _build()
    nc = _CACHE["nc"]

    xt_cache = {}
    in_maps = [
        _prep_core_inputs(c, x, Wq, bq, Wk, bk, Wv, bv, Wo, np_bf16, xt_cache)
        for c in range(NCORE)]
    res = run_bass_kernel_spmd(nc, in_maps, list(range(NCORE)))
    parts = [res.results[c]["out"].astype(np.float32) for c in range(NCORE)]
    out0 = parts[0] + parts[1] + parts[2] + parts[3] + bo
    out1 = parts[4] + parts[5] + parts[6] + parts[7] + bo
    return np.stack([out0, out1]).astype(np.float32)
